# revision 5
# baseline (speedup 1.0000x reference)
"""BertAttention (B=2, S=2048, H=1024, 16 heads) on 8 trn2 NeuronCores.

Sharding: tensor-parallel over heads (2 heads/core). Each core computes
Q/K/V projections for its 2 heads over all tokens, full attention for
those heads, then the per-head context vectors are exchanged with an
AllToAll (2MB/core) so that each core ends up with the full 1024-dim
context for a 512-token slice. The output dense + residual + LayerNorm
run locally on that token slice; the host concatenates the 8 slices.

Device layout notes:
  - All matmuls run as float32r (full PE rate at N>=512, ~fp32 precision).
  - Scores are computed transposed ([key, query] layout) so softmax's
    reduction lands on the partition axis, where it is fused into the
    P@V matmul via an appended ones-column on V (row 64 of ctx^T psum is
    the softmax denominator). exp() is safe without max-subtraction here
    (scores are O(1)); the attention mask is folded into the scores
    matmul via an extra contraction row (mask row in K, ones row in Q).
  - 1/sqrt(sqrt(d)) scaling of q and k is folded into the weights on host.
  - LayerNorm rsqrt is computed as exp(-0.5*ln(var+eps)) to stay in the
    exp/ln ACT table set (the hardware Sqrt/Rsqrt tables are imprecise).
"""

import numpy as np

import concourse.bacc as bacc
import concourse.bass as bass
import concourse.bass_utils as bass_utils
import concourse.mybir as mybir
import concourse.tile as tile
from concourse.masks import make_identity

B, S, H, NH, HD = 2, 2048, 1024, 16, 64
N_CORES = 8
HPC = NH // N_CORES          # heads per core: 2
F = HPC * HD                 # features per core: 128
TOK = B * S                  # 4096
TPC = TOK // N_CORES         # tokens per core: 512
EPS = 1e-12
_ONES = np.ones((128, 2048), dtype=np.float32)

f32 = mybir.dt.float32
f32r = mybir.dt.float32r
AF = mybir.ActivationFunctionType
ALU = mybir.AluOpType


def _build():
    nc = bacc.Bacc("TRN2", target_bir_lowering=False, debug=False,
                   enable_asserts=True, num_devices=N_CORES)

    hid_t = nc.dram_tensor("hid_t", [B * H, S], f32r, kind="ExternalInput")
    wq_t = nc.dram_tensor("wq_t", [H, F], f32r, kind="ExternalInput")
    wk_t = nc.dram_tensor("wk_t", [H, F], f32r, kind="ExternalInput")
    wv_t = nc.dram_tensor("wv_t", [H, F], f32r, kind="ExternalInput")
    bq = nc.dram_tensor("bq", [F, 1], f32, kind="ExternalInput")
    bk = nc.dram_tensor("bk", [F, 1], f32, kind="ExternalInput")
    bv = nc.dram_tensor("bv", [F, 1], f32, kind="ExternalInput")
    mask_t = nc.dram_tensor("mask_t", [B, S], f32r, kind="ExternalInput")
    wo_t = nc.dram_tensor("wo_t", [H, H], f32r, kind="ExternalInput")
    resid = nc.dram_tensor("resid", [TPC, H], f32, kind="ExternalInput")
    gamma_b = nc.dram_tensor("gamma_b", [128, H], f32, kind="ExternalInput")
    beta_b = nc.dram_tensor("beta_b", [128, H], f32, kind="ExternalInput")
    ones_d = nc.dram_tensor("ones_d", [128, S], f32r, kind="ExternalInput")
    out = nc.dram_tensor("out", [TPC, H], f32, kind="ExternalOutput")

    NSC = S // 512           # 4 s-chunks per batch
    NTB = S // 128           # 16 key blocks per batch

    with tile.TileContext(nc) as tc:
        with tc.tile_pool(name="const", bufs=1) as cst, \
             tc.tile_pool(name="w", bufs=1) as wp, \
             tc.tile_pool(name="hid", bufs=4) as hp, \
             tc.tile_pool(name="qk", bufs=1) as qp, \
             tc.tile_pool(name="e", bufs=4) as ep, \
             tc.tile_pool(name="cx", bufs=2) as cp, \
             tc.tile_pool(name="op", bufs=2) as op, \
             tc.tile_pool(name="ps_s", bufs=2, space="PSUM") as ps_s, \
             tc.tile_pool(name="ps_c", bufs=2, space="PSUM") as ps_c, \
             tc.tile_pool(name="ps_m", bufs=2, space="PSUM") as ps_m, \
             tc.tile_pool(name="dram", bufs=1, space="DRAM") as dp:

            ident = cst.tile([128, 128], f32)
            make_identity(nc, ident)

            wq_sb = wp.tile([128, 8, F], f32r, tag="wq")
            wk_sb = wp.tile([128, 8, F], f32r, tag="wk")
            wv_sb = wp.tile([128, 8, F], f32r, tag="wv")
            nc.sync.dma_start(wq_sb[:], wq_t.ap().rearrange("(hb p) f -> p hb f", p=128))
            nc.sync.dma_start(wk_sb[:], wk_t.ap().rearrange("(hb p) f -> p hb f", p=128))
            nc.sync.dma_start(wv_sb[:], wv_t.ap().rearrange("(hb p) f -> p hb f", p=128))
            wo_sb = wp.tile([128, 8, H], f32r, tag="wo")
            nc.sync.dma_start(wo_sb[:], wo_t.ap().rearrange("(fb p) o -> p fb o", p=128))

            bq_sb = cst.tile([F, 1], f32, tag="bq")
            bk_sb = cst.tile([F, 1], f32, tag="bk")
            bv_sb = cst.tile([F, 1], f32, tag="bv")
            nc.sync.dma_start(bq_sb[:], bq[:, :])
            nc.sync.dma_start(bk_sb[:], bk[:, :])
            nc.sync.dma_start(bv_sb[:], bv[:, :])
            gam_sb = cst.tile([128, H], f32, tag="gam")
            bet_sb = cst.tile([128, H], f32, tag="bet")
            nc.sync.dma_start(gam_sb[:], gamma_b[:, :])
            nc.sync.dma_start(bet_sb[:], beta_b[:, :])
            res_sb = cst.tile([128, 4, H], f32, tag="res")
            nc.sync.dma_start(res_sb[:], resid.ap().rearrange("(sb p) o -> p sb o", p=128))

            eps_sb = cst.tile([128, 1], f32, tag="eps")
            nc.vector.memset(eps_sb[:, :], EPS)

            a2a_in = dp.tile([TOK, F], f32, tag="a2a_in")
            a2a_out = dp.tile([TOK, F], f32, tag="a2a_out")

            for b in range(B):
                QT1 = [qp.tile([65, S], f32r, tag=f"qt1_{h}", name=f"qt1_{b}_{h}") for h in range(HPC)]
                KTm = [qp.tile([65, S], f32r, tag=f"ktm_{h}", name=f"ktm_{b}_{h}") for h in range(HPC)]
                Vpl = [qp.tile([128, NTB, 65], f32r, tag=f"vpl_{h}", name=f"vpl_{b}_{h}") for h in range(HPC)]
                for h in range(HPC):
                    nc.sync.dma_start(QT1[h][64:65, :], ones_d[0:1, :])
                    nc.sync.dma_start(KTm[h][64:65, :], mask_t[b:b + 1, :])
                    nc.sync.dma_start(Vpl[h][:, :, 64:65], ones_d[:, 0:NTB])

                # ---- QKV projection (streaming hidden^T chunks) ----
                for sc in range(NSC):
                    qk_ps = ps_s.tile([128, 2, 512], f32, tag="s")
                    v_ps = ps_m.tile([128, 512], f32, tag="m")
                    for hb in range(8):
                        hch = hp.tile([128, 512], f32r, tag="h")
                        nc.sync.dma_start(
                            hch[:],
                            hid_t[b * H + hb * 128:b * H + (hb + 1) * 128,
                                  sc * 512:(sc + 1) * 512])
                        rm = hch[:, :]
                        st, sp = (hb == 0), (hb == 7)
                        nc.tensor.matmul(qk_ps[:, 0:1, :], wq_sb[:, hb:hb + 1, :],
                                         rm, start=st, stop=sp, skip_group_check=True)
                        nc.tensor.matmul(qk_ps[:, 1:2, :], wk_sb[:, hb:hb + 1, :],
                                         rm, start=st, stop=sp, skip_group_check=True)
                        nc.tensor.matmul(v_ps[:, :], wv_sb[:, hb:hb + 1, :],
                                         rm, start=st, stop=sp, skip_group_check=True)
                    for h in range(HPC):
                        r0, r1 = h * HD, (h + 1) * HD
                        nc.vector.tensor_scalar_add(
                            QT1[h][0:HD, sc * 512:(sc + 1) * 512],
                            qk_ps[r0:r1, 0:1, :], bq_sb[r0:r1, :])
                        nc.vector.tensor_scalar_add(
                            KTm[h][0:HD, sc * 512:(sc + 1) * 512],
                            qk_ps[r0:r1, 1:2, :], bk_sb[r0:r1, :])
                    vt = qp.tile([128, 512], f32, tag="vt")
                    nc.vector.tensor_scalar_add(vt[:, :], v_ps[:, :], bv_sb[:, :])
                    for j in range(4):
                        tb = sc * 4 + j
                        tp = ps_m.tile([128, 512], f32, tag="m")
                        nc.tensor.transpose(tp[:, 0:128], vt[:, j * 128:(j + 1) * 128],
                                            ident[:, :])
                        for h in range(HPC):
                            nc.vector.tensor_copy(Vpl[h][:, tb:tb + 1, 0:HD],
                                                  tp[:, h * HD:(h + 1) * HD])

                # ---- attention ----
                for sc in range(NSC):
                    ctxT = []
                    for h in range(HPC):
                        ctx_ps = ps_c.tile([65, 512], f32, tag="c")
                        for tbp in range(NTB // 2):
                            sc_ps = ps_s.tile([128, 2, 512], f32, tag="s")
                            e_sb = ep.tile([128, 2, 512], f32r, tag="e")
                            for i in range(2):
                                tb = tbp * 2 + i
                                nc.tensor.matmul(
                                    sc_ps[:, i:i + 1, :],
                                    KTm[h][:, tb * 128:(tb + 1) * 128],
                                    QT1[h][:, sc * 512:(sc + 1) * 512],
                                    start=True, stop=True, skip_group_check=True)
                            nc.scalar.activation(e_sb[:, :, :], sc_ps[:, :, :], AF.Exp)
                            for i in range(2):
                                tb = tbp * 2 + i
                                nc.tensor.matmul(
                                    ctx_ps[:, :],
                                    Vpl[h][:, tb:tb + 1, :],
                                    e_sb[:, i:i + 1, :],
                                    start=(tb == 0), stop=(tb == NTB - 1),
                                    skip_group_check=True)
                        ct = cp.tile([65, 512], f32, tag=f"ctxT_{h}")
                        nc.vector.tensor_copy(ct[:, :], ctx_ps[:, :])
                        ctxT.append(ct)
                    for sb in range(4):
                        tp = ps_m.tile([128, 512], f32, tag="m")
                        r = cp.tile([128, 2], f32, tag="r")
                        stg = cp.tile([128, F], f32, tag="stg")
                        for h in range(HPC):
                            nc.tensor.transpose(
                                tp[:, h * 65:(h + 1) * 65],
                                ctxT[h][:, sb * 128:(sb + 1) * 128],
                                ident[0:65, 0:65])
                            nc.vector.reciprocal(r[:, h:h + 1],
                                                 tp[:, h * 65 + 64:h * 65 + 65])
                            nc.vector.tensor_scalar_mul(
                                stg[:, h * HD:(h + 1) * HD],
                                tp[:, h * 65:h * 65 + HD], r[:, h:h + 1])
                        row = b * S + sc * 512 + sb * 128
                        nc.sync.dma_start(a2a_in[row:row + 128, :], stg[:, :])

            # ---- exchange context slices ----
            nc.gpsimd.collective_compute(
                "AllToAll", ALU.bypass,
                replica_groups=[list(range(N_CORES))],
                ins=[a2a_in.opt()], outs=[a2a_out.opt()])

            # ---- output dense + residual + LayerNorm on my 512 tokens ----
            recvT = op.tile([128, 32, 128], f32r, tag="recvT")
            for j in range(N_CORES):
                for sb in range(4):
                    rin = hp.tile([128, 128], f32, tag="rin")
                    row = j * TPC + sb * 128
                    nc.sync.dma_start(rin[:], a2a_out[row:row + 128, :])
                    tp = ps_m.tile([128, 512], f32, tag="m")
                    nc.tensor.transpose(tp[:, 0:128], rin[:, :], ident[:, :])
                    nc.vector.tensor_copy(recvT[:, (j * 4 + sb):(j * 4 + sb) + 1, :],
                                          tp[:, 0:128])
            for sb in range(4):
                o_ps = ps_s.tile([128, 2, 512], f32, tag="s")
                for oc in range(2):
                    for j in range(N_CORES):
                        nc.tensor.matmul(
                            o_ps[:, oc:oc + 1, :],
                            recvT[:, (j * 4 + sb):(j * 4 + sb) + 1, :],
                            wo_sb[:, j:j + 1, oc * 512:(oc + 1) * 512],
                            start=(j == 0), stop=(j == N_CORES - 1),
                            skip_group_check=True)
                x = op.tile([128, H], f32, tag="x")
                acc = op.tile([128, 2], f32, tag="acc")
                for oc in range(2):
                    nc.vector.scalar_tensor_tensor(
                        out=x[:, oc * 512:(oc + 1) * 512],
                        in0=o_ps[:, oc:oc + 1, :], scalar=1.0,
                        in1=res_sb[:, sb:sb + 1, oc * 512:(oc + 1) * 512],
                        op0=ALU.mult, op1=ALU.add,
                        accum_out=acc[:, oc:oc + 1])
                mean = op.tile([128, 1], f32, tag="mean")
                nc.vector.tensor_add(mean[:, :], acc[:, 0:1], acc[:, 1:2])
                nc.vector.tensor_scalar_mul(mean[:, :], mean[:, :], 1.0 / H)
                xc = op.tile([128, H], f32, tag="xc")
                nc.vector.tensor_scalar_sub(xc[:, :], x[:, :], mean[:, :])
                vacc = op.tile([128, 1], f32, tag="vacc")
                nc.vector.scalar_tensor_tensor(
                    out=x[:, :], in0=xc[:, :], scalar=1.0, in1=xc[:, :],
                    op0=ALU.mult, op1=ALU.mult, accum_out=vacc[:, :])
                lnv = op.tile([128, 1], f32, tag="lnv")
                nc.scalar.activation(lnv[:, :], vacc[:, :], AF.Ln,
                                     bias=eps_sb[:, :], scale=1.0 / H)
                rstd = op.tile([128, 1], f32, tag="rstd")
                nc.scalar.activation(rstd[:, :], lnv[:, :], AF.Exp, scale=-0.5)
                y = op.tile([128, H], f32, tag="y")
                nc.vector.scalar_tensor_tensor(
                    out=y[:, :], in0=xc[:, :], scalar=rstd[:, :], in1=gam_sb[:, :],
                    op0=ALU.mult, op1=ALU.mult)
                nc.vector.tensor_add(y[:, :], y[:, :], bet_sb[:, :])
                nc.sync.dma_start(out[sb * 128:(sb + 1) * 128, :], y[:, :])

    nc.compile()
    return nc


_NC_CACHE = []


def _get_nc():
    if not _NC_CACHE:
        _NC_CACHE.append(_build())
    return _NC_CACHE[0]


def _prep_in_maps(hidden_states, attention_mask, W_qkv, b_qkv, W_out, b_out,
                  ln_gamma, ln_beta):
    hidden_states = np.asarray(hidden_states, dtype=np.float32)
    attention_mask = np.asarray(attention_mask, dtype=np.float32)
    W_qkv = np.asarray(W_qkv, dtype=np.float32)
    b_qkv = np.asarray(b_qkv, dtype=np.float32)
    W_out = np.asarray(W_out, dtype=np.float32)
    b_out = np.asarray(b_out, dtype=np.float32)
    ln_gamma = np.asarray(ln_gamma, dtype=np.float32)
    ln_beta = np.asarray(ln_beta, dtype=np.float32)

    qk_scale = 1.0 / np.sqrt(np.sqrt(np.float32(HD)))          # 1/d^0.25
    sc2 = np.float32(qk_scale * qk_scale)                      # applied to q,k product

    hid_t = np.ascontiguousarray(
        hidden_states.transpose(0, 2, 1).reshape(B * H, S))
    mask_t = np.ascontiguousarray(attention_mask.reshape(B, S))
    hid_flat = hidden_states.reshape(TOK, H)
    gamma_b = np.ascontiguousarray(np.broadcast_to(ln_gamma, (128, H)))
    beta_b = np.ascontiguousarray(np.broadcast_to(ln_beta, (128, H)))

    in_maps = []
    for c in range(N_CORES):
        r0 = c * F
        wq = W_qkv[r0:r0 + F, :]
        wk = W_qkv[H + r0:H + r0 + F, :]
        wv = W_qkv[2 * H + r0:2 * H + r0 + F, :]
        in_maps.append({
            "hid_t": hid_t,
            "wq_t": np.ascontiguousarray(wq.T * sc2),
            "wk_t": np.ascontiguousarray(wk.T),
            "wv_t": np.ascontiguousarray(wv.T),
            "bq": np.ascontiguousarray((b_qkv[r0:r0 + F] * sc2).reshape(F, 1)),
            "bk": np.ascontiguousarray(b_qkv[H + r0:H + r0 + F].reshape(F, 1)),
            "bv": np.ascontiguousarray(b_qkv[2 * H + r0:2 * H + r0 + F].reshape(F, 1)),
            "mask_t": mask_t,
            "wo_t": np.ascontiguousarray(W_out.T),
            "resid": np.ascontiguousarray(hid_flat[c * TPC:(c + 1) * TPC, :] + b_out),
            "gamma_b": gamma_b,
            "beta_b": beta_b,
            "ones_d": _ONES,
        })
    return in_maps


def run(trace=False, **inputs):
    nc = _get_nc()
    in_maps = _prep_in_maps(**inputs)
    res = bass_utils.run_bass_kernel_spmd(
        nc, in_maps, core_ids=list(range(N_CORES)), trace=trace)
    out = np.concatenate([res.results[c]["out"] for c in range(N_CORES)], axis=0)
    return out.reshape(B, S, H).astype(np.float32), res


def kernel(**inputs):
    out, _ = run(trace=False, **inputs)
    return out
